# revision 19
# baseline (speedup 1.0000x reference)
"""Criss-cross attention (2-stream) Trainium2 kernel.

Data-parallel over batch B=8 across 8 NeuronCores; one image pair per core.

Per-core algorithm (all matmuls bf16, fp32 PSUM accumulation):
  - q/k projections for both streams in one pass (bias host-corrected,
    applied on ScalarE). k/q stored in two [64,S] tiles so each attend's
    (k, q) pair sits at the same base partition (rows 0:32 = attend1 pair,
    rows 32:64 = attend0 pair) -> no per-attend SBUF copies, and the two
    attends' logit matmuls alternate PE row-groups.
  - diagonal mask pre-accumulated into PSUM via an identity-stationary
    matmul; column logit matmuls accumulate on top; joint softmax
    without max-subtraction (logits are O(30); exp safe in f32).
  - row logits first; column logits pipelined per 4-w group with the
    softmax-denominator stats (PE ones-matmul -> DVE fast reciprocal ->
    GpSimd copy to bf16) and P-normalization (DVE, in place). No
    activation-table thrash: ScalarE runs Exp/Copy/Identity only.
  - Z-trick: Z[c',p] = sum_g x[c',g] * Phat[p,g] using host-supplied
    spatially-transposed x copies, then one dense (gamma*wv) @ Z projection
    with the residual x~ = x + gamma*bv accumulated into the same PSUM via
    an identity-stationary matmul; PSUM->SBUF moves alternate ScalarE/DVE.
    v-bias folds out exactly because joint softmax weights sum to 1 (bq' =
    bq - wq@(gamma*bv), bk' = bk - wk@(gamma*bv) correct the projections).
  - the two attends are pipeline-interleaved:
      R0 R1 C0 Zc0 C1 Zr0 F0 Zc1 Zr1 F1
    so ACT-heavy (exp, PSUM->SBUF copies) and DVE-heavy (row-accum,
    reciprocal) phases overlap across attends.
"""

import sys

sys.path.insert(0, "/opt/trn_rl_repo")

import numpy as np
import ml_dtypes

BF = ml_dtypes.bfloat16
B, C, H, W = 8, 256, 96, 96
CQ = 32
S = H * W  # 9216
NSL = S // 512  # 18
NEG = -1.0e30
GRP = 4   # logit slices per et tile (one psum bank)
SLW = 8   # spatial slices per xT slab

_CACHE = {}


def build_nc(reps=1):
    import concourse.tile as tile
    from concourse import bacc, mybir

    f32 = mybir.dt.float32
    bf16 = mybir.dt.bfloat16

    nc = bacc.Bacc("TRN2", target_bir_lowering=False, debug=False, num_devices=8)

    din = {}

    def dparam(name, shape, dt=bf16):
        din[name] = nc.dram_tensor(name, shape, dt, kind="ExternalInput").ap()

    dparam("xa0", [C, S])          # bf16(x0 + g*bv0), channel-major
    dparam("xa1", [C, S])
    dparam("xtc0", [H, W * C])     # xtc[h, w*256+c] = x0[c,h,w]  (raw x)
    dparam("xtr0", [W, H * C])     # xtr[w, h*256+c] = x0[c,h,w]
    dparam("xtc1", [H, W * C])
    dparam("xtr1", [W, H * C])
    dparam("wqk", [C, 128])        # cols: wk1T|wk0T|wq0T|wq1T
    dparam("wv0", [C, C])          # (gamma*wv0).T
    dparam("wv1", [C, C])
    dparam("qkbK", [64, 1], f32)   # bk1'|bk0'
    dparam("qkbQ", [64, 1], f32)   # bq0'|bq1'
    dparam("maskrep", [H, 4 * H])  # -1e30 on (h', j*96+h'), else 0 (bf16)
    dparam("eye", [128, 128])      # identity (bf16)
    out = nc.dram_tensor("out", [2, C, S], bf16, kind="ExternalOutput").ap()

    with tile.TileContext(nc) as tc:
        if reps == 1:
            _emit(tc, nc, din, out, mybir)
        else:
            with tc.For_i(0, reps, 1):
                _emit(tc, nc, din, out, mybir)

    nc.compile()
    return nc


def _emit(tc, nc, din, out, mybir):
    from contextlib import ExitStack

    f32 = mybir.dt.float32
    bf16 = mybir.dt.bfloat16
    EXP = mybir.ActivationFunctionType.Exp
    CPY = mybir.ActivationFunctionType.Copy
    IDT = mybir.ActivationFunctionType.Identity
    ADD = mybir.AluOpType.add
    MUL = mybir.AluOpType.mult

    SPAD = 128  # free-dim pad on tile_K for 128-col (FWL) row-logit weights

    ctx = ExitStack()
    with ctx:
        const = ctx.enter_context(tc.tile_pool(name="const", bufs=1))
        persist = ctx.enter_context(tc.tile_pool(name="persist", bufs=1))
        slab = ctx.enter_context(tc.tile_pool(name="slab", bufs=3))
        rrp = ctx.enter_context(tc.tile_pool(name="rrp", bufs=2))
        obuf = ctx.enter_context(tc.tile_pool(name="obuf", bufs=2))
        resl = ctx.enter_context(tc.tile_pool(name="resl", bufs=2))
        # PSUM pools: et0+et1 (1 bank each) + lps 2 + ps 4 = 8 banks
        eps = ctx.enter_context(tc.tile_pool(name="eps", bufs=1, space="PSUM"))
        lps = ctx.enter_context(tc.tile_pool(name="lps", bufs=2, space="PSUM"))
        ps = ctx.enter_context(tc.tile_pool(name="ps", bufs=4, space="PSUM"))

        # ---------------- constants ----------------
        wqk_t = []
        for kc in range(2):
            t = const.tile([128, 128], bf16, tag=f"wqk{kc}", name=f"wqk{kc}")
            nc.scalar.dma_start(t[:], din["wqk"][kc * 128:(kc + 1) * 128, :])
            wqk_t.append(t)
        wv_t = [[None] * 2 for _ in range(2)]
        for s in range(2):
            for kc in range(2):
                t = const.tile([128, 256], bf16, tag=f"wv{s}{kc}",
                               name=f"wv{s}{kc}")
                nc.scalar.dma_start(
                    t[:], din[f"wv{s}"][kc * 128:(kc + 1) * 128, :]
                )
                wv_t[s][kc] = t
        qkbK_t = const.tile([64, 1], f32, tag="qkbK")
        nc.scalar.dma_start(qkbK_t[:], din["qkbK"][:])
        qkbQ_t = const.tile([64, 1], f32, tag="qkbQ")
        nc.scalar.dma_start(qkbQ_t[:], din["qkbQ"][:])
        mrep_t = const.tile([H, 4 * H], bf16, tag="mrep")
        nc.scalar.dma_start(mrep_t[:], din["maskrep"][:])
        eye_t = const.tile([128, 128], bf16, tag="eye")
        nc.scalar.dma_start(eye_t[:], din["eye"][:])
        ones_t = const.tile([H, H], bf16, tag="ones")
        nc.vector.memset(ones_t[:], 1.0)

        # ---------------- q/k projections ----------------
        # tile_K rows: k1(0:32) k0(32:64); tile_Q rows: q0(0:32) q1(32:64)
        # attend a=0 pair (k0,q1) at base 32; a=1 pair (k1,q0) at base 0.
        tile_K = persist.tile([64, S + SPAD], bf16, tag="tileK", name="tile_K")
        tile_Q = persist.tile([64, S], bf16, tag="tileQ", name="tile_Q")
        nc.vector.memset(tile_K[:][:, S:S + SPAD], 0.0)
        for n0 in range(0, NSL, 2):
            xsl = [[None] * 2 for _ in range(2)]
            for s in range(2):
                for kc in range(2):
                    t = resl.tile([128, 1024], bf16, tag=f"x{s}s{kc}",
                                  name=f"x{s}s{kc}")
                    nc.sync.dma_start(
                        t[:],
                        din[f"xa{s}"][kc * 128:(kc + 1) * 128,
                                      n0 * 512:(n0 + 2) * 512],
                    )
                    xsl[s][kc] = t
            for j in range(2):
                sl = slice((n0 + j) * 512, (n0 + j + 1) * 512)
                jsl = slice(j * 512, (j + 1) * 512)
                pK = ps.tile([64, 512], f32, tag="ps", name="pK")
                pQ = ps.tile([64, 512], f32, tag="ps", name="pQ")
                # k1 from x1, k0 from x0; q0 from x0, q1 from x1
                for kc in range(2):
                    nc.tensor.matmul(
                        pK[0:32, :], wqk_t[kc][:, 0:32], xsl[1][kc][:, jsl],
                        start=(kc == 0), stop=(kc == 1),
                    )
                for kc in range(2):
                    nc.tensor.matmul(
                        pK[32:64, :], wqk_t[kc][:, 32:64], xsl[0][kc][:, jsl],
                        start=(kc == 0), stop=(kc == 1),
                        tile_position=(0, 32), skip_group_check=True,
                    )
                for kc in range(2):
                    nc.tensor.matmul(
                        pQ[0:32, :], wqk_t[kc][:, 64:96], xsl[0][kc][:, jsl],
                        start=(kc == 0), stop=(kc == 1), skip_group_check=True,
                    )
                for kc in range(2):
                    nc.tensor.matmul(
                        pQ[32:64, :], wqk_t[kc][:, 96:128], xsl[1][kc][:, jsl],
                        start=(kc == 0), stop=(kc == 1),
                        tile_position=(0, 32), skip_group_check=True,
                    )
                nc.scalar.activation(tile_K[:][:, sl], pK[:], IDT, bias=qkbK_t[:])
                nc.scalar.activation(tile_Q[:][:, sl], pQ[:], IDT, bias=qkbQ_t[:])

        K_wh = tile_K[:][:, 0:S].rearrange("p (h w) -> p w h", w=W)
        Q_wh = tile_Q[:].rearrange("p (h w) -> p w h", w=W)

        # per-attend row slices: a=0 -> 32:64, a=1 -> 0:32
        arows = [slice(32, 64), slice(0, 32)]

        pcol = [None, None]
        prow = [None, None]
        prow_hw = [None, None]
        for a in range(2):
            pcol[a] = persist.tile([H, S], bf16, tag=f"pcol{a}", name=f"pcol{a}")
            prow[a] = persist.tile([W, S], bf16, tag=f"prow{a}", name=f"prow{a}")
            prow_hw[a] = prow[a][:].rearrange("p (w h) -> p h w", h=H)
        rrep = persist.tile([H, S], bf16, tag="rrep", name="rrep")
        # pixel (h,w) lives at free index w*96+h in pcol/prow/rrep

        # ---------------- R: row logits + exp (both attends) -------------
        # per h, e[w',w] = sum_c k[c,h,w'] q[c,h,w]  (no mask)
        for h0 in range(0, H, GRP):
            et = [eps.tile([128, 512], f32, tag=f"et{a}", name=f"et{a}")
                  for a in range(2)]
            for j in range(GRP):
                h = h0 + j
                for a in range(2):
                    nc.tensor.matmul(
                        et[a][:, j * 96:(j + 1) * 96],
                        tile_K[arows[a], h * 96:h * 96 + 128],
                        tile_Q[arows[a], h * 96:(h + 1) * 96],
                        start=True, stop=True, skip_group_check=True,
                    )
            for a in range(2):
                nc.scalar.activation(
                    prow_hw[a][:, h0:h0 + GRP, :], et[a][0:96, 0:384], EXP
                )

        # ------- C(a): col logits+mask+exp pipelined with stats+norm ------
        def colpipe(a):
            for g in range(W // GRP):
                w0 = g * GRP
                gsl = slice(w0 * 96, (w0 + GRP) * 96)
                et = eps.tile([128, 512], f32, tag=f"et{a}", name=f"et{a}")
                nc.tensor.matmul(
                    et[0:96, 0:384], eye_t[0:96, 0:96], mrep_t[:],
                    start=True, stop=False, skip_group_check=True,
                )
                for j in range(GRP):
                    w = w0 + j
                    nc.tensor.matmul(
                        et[0:96, j * 96:(j + 1) * 96],
                        K_wh[arows[a], w, :],
                        Q_wh[arows[a], w, :],
                        start=False, stop=True, skip_group_check=True,
                    )
                nc.scalar.activation(pcol[a][:, gsl], et[0:96, 0:384], EXP)
                # stats for this 384-col slice: l = colsum + rowsum; 1/l
                lt = lps.tile([96, 384], f32, tag="lps", name="lt")
                nc.tensor.matmul(
                    lt[:], ones_t[:], pcol[a][:, gsl],
                    start=True, stop=False, skip_group_check=True,
                )
                nc.tensor.matmul(
                    lt[:], ones_t[:], prow[a][:, gsl],
                    start=False, stop=True, skip_group_check=True,
                )
                rr = rrp.tile([96, 384], f32, tag="rr", name="rr")
                nc.vector.reciprocal_approx_fast(rr[:], lt[:])
                nc.gpsimd.tensor_copy(rrep[:, gsl], rr[:])
                if g % 3 == 2:
                    csl = slice((w0 - 8) * 96, (w0 + GRP) * 96)
                    nc.vector.tensor_tensor(
                        pcol[a][:, csl], pcol[a][:, csl], rrep[:, csl], MUL
                    )
                    nc.gpsimd.tensor_tensor(
                        prow[a][:, csl], prow[a][:, csl], rrep[:, csl], MUL
                    )

        z = [[None, None], [None, None]]

        # attend0's z lives in tags z0/z1; attend1's z reuses the dead
        # pcol0/prow0 buffers so Zc1/Zr1 need not wait for F0's z reads.
        ztags = [("z0", "z1"), ("pcol0", "prow0")]

        def zcol(a):
            # column branch: per w, Z[c', h]; scatter w-strided into z
            for kc in range(2):
                z[a][kc] = persist.tile([128, S], bf16, tag=ztags[a][kc],
                                        name=f"z{a}{kc}")
            z_wh = [zz[:].rearrange("p (h w) -> p w h", w=W) for zz in z[a]]
            xtc = din[f"xtc{a}"][:].rearrange("p (w c) -> p w c", c=C)
            for w0 in range(0, W, SLW):
                xs = slab.tile([H, SLW * 256], bf16, tag="xslab", name="xsc")
                nc.sync.dma_start(xs[:], xtc[:, w0:w0 + SLW, :])
                for kc in range(2):
                    for j0 in range(0, SLW, 4):
                        zp = ps.tile([128, 512], f32, tag="ps", name="zp")
                        for j in range(4):
                            wl = j0 + j
                            nc.tensor.matmul(
                                zp[:, j * 96:(j + 1) * 96],
                                xs[:, wl * 256 + kc * 128:wl * 256 + kc * 128 + 128],
                                pcol[a][:, (w0 + wl) * 96:(w0 + wl + 1) * 96],
                                start=True, stop=True, skip_group_check=True,
                            )
                        nc.scalar.activation(
                            z_wh[kc][:, w0 + j0:w0 + j0 + 4, :],
                            zp[:, 0:384], CPY,
                        )

        def zrow(a):
            # row branch: per h, Z[c', w]; accumulate into z
            xtr = din[f"xtr{a}"][:].rearrange("p (h c) -> p h c", c=C)
            for h0 in range(0, H, SLW):
                xs = slab.tile([W, SLW * 256], bf16, tag="xslab", name="xsr")
                nc.sync.dma_start(xs[:], xtr[:, h0:h0 + SLW, :])
                for kc in range(2):
                    for j0 in range(0, SLW, 4):
                        zp = ps.tile([128, 512], f32, tag="ps", name="zp")
                        for j in range(4):
                            hl = j0 + j
                            nc.tensor.matmul(
                                zp[:, j * 96:(j + 1) * 96],
                                xs[:, hl * 256 + kc * 128:hl * 256 + kc * 128 + 128],
                                prow_hw[a][:, h0 + hl, :],
                                start=True, stop=True, skip_group_check=True,
                            )
                        zsl = z[a][kc][:, (h0 + j0) * 96:(h0 + j0 + 4) * 96]
                        nc.vector.tensor_tensor(zsl, zp[:, 0:384], zsl, ADD)

        def final(a):
            # final projection + residual (identity-matmul) + store;
            # PSUM->SBUF moves alternate ScalarE / DVE.
            for mc in range(2):
                for n0 in range(0, NSL, 2):
                    ob = obuf.tile([128, 1024], bf16, tag="ob", name="ob")
                    rt = resl.tile([128, 1024], bf16, tag="res", name="res")
                    nc.sync.dma_start(
                        rt[:],
                        din[f"xa{a}"][mc * 128:(mc + 1) * 128,
                                      n0 * 512:(n0 + 2) * 512],
                    )
                    for j in range(2):
                        n = n0 + j
                        sl = slice(n * 512, (n + 1) * 512)
                        jsl = slice(j * 512, (j + 1) * 512)
                        op = ps.tile([128, 512], f32, tag="ps", name="op")
                        for kc in range(2):
                            nc.tensor.matmul(
                                op[:],
                                wv_t[a][kc][:, mc * 128:(mc + 1) * 128],
                                z[a][kc][:, sl],
                                start=(kc == 0), stop=False,
                            )
                        nc.tensor.matmul(
                            op[:], eye_t[:], rt[:, jsl],
                            start=False, stop=True, skip_group_check=True,
                        )
                        if j == 0:
                            nc.scalar.activation(ob[:, jsl], op[:], CPY)
                        else:
                            nc.vector.tensor_copy(ob[:, jsl], op[:])
                    nc.gpsimd.dma_start(
                        out[a, mc * 128:(mc + 1) * 128,
                            n0 * 512:(n0 + 2) * 512],
                        ob[:],
                    )

        # interleaved schedule: C1's ACT work (exps) overlaps Zr0's DVE
        # work; Zc1's ACT copies overlap F0's PE/DVE work.
        colpipe(0)
        zcol(0)
        zrow(0)
        colpipe(1)
        zcol(1)
        final(0)
        zrow(1)
        final(1)


def prep_inputs(inputs):
    """Host-side per-core input prep (numpy)."""
    g = float(np.asarray(inputs["gamma"]).reshape(-1)[0])
    mask = np.zeros((H, H), np.float32)
    np.fill_diagonal(mask, NEG)
    maskrep = np.tile(mask, (1, 4)).astype(BF)
    eye = np.eye(128, dtype=np.float32).astype(BF)
    wqk = np.concatenate(
        [inputs["wk1"].T, inputs["wk0"].T, inputs["wq0"].T, inputs["wq1"].T],
        axis=1,
    ).astype(BF)
    wv0 = (g * np.asarray(inputs["wv0"], np.float64)).T.astype(BF)
    wv1 = (g * np.asarray(inputs["wv1"], np.float64)).T.astype(BF)
    gb0 = g * np.asarray(inputs["bv0"], np.float64)
    gb1 = g * np.asarray(inputs["bv1"], np.float64)
    qkbK = np.concatenate(
        [
            inputs["bk1"] - inputs["wk1"].astype(np.float64) @ gb1,
            inputs["bk0"] - inputs["wk0"].astype(np.float64) @ gb0,
        ]
    ).astype(np.float32)[:, None]
    qkbQ = np.concatenate(
        [
            inputs["bq0"] - inputs["wq0"].astype(np.float64) @ gb0,
            inputs["bq1"] - inputs["wq1"].astype(np.float64) @ gb1,
        ]
    ).astype(np.float32)[:, None]
    maps = []
    for b in range(B):
        x0 = np.asarray(inputs["x0"][b], np.float32)
        x1 = np.asarray(inputs["x1"][b], np.float32)
        maps.append({
            "xa0": (x0 + np.float32(gb0[:, None, None])).reshape(C, S).astype(BF),
            "xa1": (x1 + np.float32(gb1[:, None, None])).reshape(C, S).astype(BF),
            "xtc0": np.ascontiguousarray(x0.transpose(1, 2, 0)).reshape(H, W * C).astype(BF),
            "xtr0": np.ascontiguousarray(x0.transpose(2, 1, 0)).reshape(W, H * C).astype(BF),
            "xtc1": np.ascontiguousarray(x1.transpose(1, 2, 0)).reshape(H, W * C).astype(BF),
            "xtr1": np.ascontiguousarray(x1.transpose(2, 1, 0)).reshape(W, H * C).astype(BF),
            "wqk": wqk, "wv0": wv0, "wv1": wv1,
            "qkbK": qkbK, "qkbQ": qkbQ,
            "maskrep": maskrep, "eye": eye,
        })
    return maps


def postprocess(results):
    cat0 = np.empty((B, C, H, W), np.float32)
    cat1 = np.empty((B, C, H, W), np.float32)
    for b in range(B):
        o = np.asarray(results[b]["out"]).astype(np.float32).reshape(2, C, H, W)
        cat0[b] = o[0]
        cat1[b] = o[1]
    return (cat0, cat1)


def kernel(**inputs):
    from concourse.bass_utils import run_bass_kernel_spmd

    if "nc" not in _CACHE:
        _CACHE["nc"] = build_nc()
    nc = _CACHE["nc"]
    maps = prep_inputs(inputs)
    res = run_bass_kernel_spmd(nc, maps, core_ids=list(range(B)))
    return postprocess(res.results)
